# revision 12
# baseline (speedup 1.0000x reference)
"""CrossModalAttention kernel for 8 Trainium2 NeuronCores.

Data-parallel over batch: B=16 -> 2 batches per core.

Math (per batch, with A=audio [N,d], B=video [M,d]):
  scores*sqrt(d) = (A Wa^T + 1 b_a^T)(B Wv^T + 1 b_v^T)^T
                 = A M B^T + (row-constant terms) + 1_n w^T
  with M = Wa^T Wv, w = B (Wv^T b_a).  Row-constant terms drop inside
  softmax, and max-subtraction is skipped: scores are ~N(0,1), far from
  fp16/fp32 exp overflow.
  attn     = exp_s / rowsum, with exp_s kept transposed [m, n]
  att_T[d,n] = sum_m video[m,d] exp_s[m,n]
  out[n,f] = (att_T^T @ Wo^T) * (1/rowsum[n]) + b_o

All matmul operands are fp16 (1 cyc/row on PE like bf16, fp32 PSUM
accumulation, 8x less rounding than bf16); softmax internals stay fp32.
K is always on partitions.  All transposes run on the DMA XBAR (2-byte
dtype, ~14ns per 16x128 tile), so the PE does zero transpose work.
Video is prepped in 4 groups of 4 m-chunks, each group an independent
load->cast->XBAR chain (tile-granular WAR deps would serialize a
monolithic chain).  Batch-0 video / audio-nt0 loads issue BEFORE the
weight-setup compute: the ACT sequencer is in-order, so any setup wait
would stall the load issue behind it.

  aT[d1,n]  <- one XBAR transpose of the [128, 4*512] fp16 audio tile
  gT[d2,n]  =  M[d1,d2](st)       x aT(mv)
  sT[m,n]   =  videoT[d2,m](st)   x gT(mv);   exp on ACT (fp16 out)
  acc[p,n]  += exp[p + 128*mc, n]             (DVE partial rowsum)
  rs[n,1]   =  acc[p,nslice](st)  x ones[p,1](mv)   (N=1 matmul)
  attT[d,n] =  video_r[m,d](st)   x exp(mv)
  out[n,f]  =  attT[e,n](st)      x WoT[e,f](mv); *recip on ACT evict
"""

import os
from contextlib import ExitStack

import numpy as np

# Stage bisect: 1=setup+copyout, 2=+audio prep+gT, 3=+scores/exp/rs,
# 4=full (default)
KMODE = int(os.environ.get("KMODE", "4"))

import concourse.bass as bass
import concourse.mybir as mybir
import concourse.tile as tile
from concourse import bacc
from concourse.bass_utils import run_bass_kernel_spmd

B, SEQ, D = 16, 2048, 512
NCORES = 8
BL = B // NCORES          # batches per core
P = 128
DC = D // P               # 4 chunks of the model dim
MC = SEQ // P             # 16 m-chunks per batch
VG = 4                    # video prep groups
MG = MC // VG             # m-chunks per group
NTW = 512                 # n-tile width
NT = SEQ // NTW           # 4 n-tiles per batch
NSC = NTW // P            # 4 n-subchunks per n-tile
SCALE = 1.0 / float(np.sqrt(D))

F32 = mybir.dt.float32
F16 = mybir.dt.float16
FR = mybir.dt.float32r


def _body(tc, ctx, has_ba=False):
    nc = tc.nc
    audio = nc.t_audio.ap()
    video = nc.t_video.ap()
    out = nc.t_out.ap()

    const = ctx.enter_context(tc.tile_pool(name="const", bufs=1))
    ps_big = ctx.enter_context(tc.tile_pool(name="ps_big", bufs=6, space="PSUM"))
    ps_rs = ctx.enter_context(tc.tile_pool(name="ps_rs", bufs=2, space="PSUM"))
    setup = ctx.enter_context(tc.tile_pool(name="setup", bufs=1))
    vid = ctx.enter_context(tc.tile_pool(name="vid", bufs=2))
    vraw = ctx.enter_context(tc.tile_pool(name="vraw", bufs=4))
    araw = ctx.enter_context(tc.tile_pool(name="araw", bufs=4))
    acast = ctx.enter_context(tc.tile_pool(name="acast", bufs=2))
    nt_pool = ctx.enter_context(tc.tile_pool(name="nt", bufs=2))
    exp_pool = ctx.enter_context(tc.tile_pool(name="expp", bufs=2))
    accp = ctx.enter_context(tc.tile_pool(name="accp", bufs=2))
    outp = ctx.enter_context(tc.tile_pool(name="outp", bufs=4))
    small = ctx.enter_context(tc.tile_pool(name="small", bufs=2))

    # ---- constants (no input deps) ----
    ones_f32 = const.tile([P, P], F32, tag="ones_f32")
    nc.gpsimd.memset(ones_f32[:], 1.0)
    ones_col = const.tile([P, 1], F16, tag="ones_col")
    nc.vector.tensor_copy(ones_col[:], ones_f32[:, 0:1])
    ones_row = const.tile([1, P], F16, tag="ones_row")
    nc.vector.tensor_copy(ones_row[:], ones_f32[0:1, :])

    m_sb = const.tile([P, DC, D], F16, tag="m_sb")
    woT = const.tile([P, DC, DC, P], F16, tag="woT")
    bo_bc = const.tile([P, D], F32, tag="bo_bc")
    q_bc = const.tile([P, D], F32, tag="q_bc") if has_ba else None

    # ---- issue weight DMAs early (sync HWDGE; independent of loads) ----
    wa_sb = setup.tile([P, DC, D], F32, tag="wa_sb")
    wv_sb = setup.tile([P, DC, D], F32, tag="wv_sb")
    wo_sb = setup.tile([P, DC, D], F32, tag="wo_sb")
    bo_sb = setup.tile([1, D], F32, tag="bo_sb")
    nc.sync.dma_start(wa_sb[:], nc.t_wa.ap().rearrange("(c p) d -> p c d", p=P))
    nc.sync.dma_start(wv_sb[:], nc.t_wv.ap().rearrange("(c p) d -> p c d", p=P))
    nc.sync.dma_start(wo_sb[:], nc.t_wo.ap().rearrange("(c p) d -> p c d", p=P))
    nc.sync.dma_start(bo_sb[:], nc.t_bo.ap().rearrange("(o d) -> o d", o=1))

    # ---- video prep: 4 groups, each an independent load/cast/XBAR chain ----
    def prep_video(b):
        b0 = b * SEQ
        vrg, vtg = [], []
        if has_ba:
            w_col = vid.tile([P, MC, 1], F32, tag="w_col")
            wsc = vid.tile([P, D], F32, tag="wsc")
        else:
            w_col = None
        for g in range(VG):
            vr_g = vid.tile([P, MG, D], F16, tag=f"vr{g}")
            vt_g = vid.tile([P, MG, DC, P], F16, tag=f"vt{g}")
            for j in range(MG):
                mc = g * MG + j
                vr = vraw.tile([P, D], F32, tag="vraw")
                nc.scalar.dma_start(vr[:], video[b0 + mc * P:b0 + (mc + 1) * P, :])
                nc.vector.tensor_copy(vr_g[:, j, :], vr[:])
                if has_ba:
                    # w = (video @ q)*scale via mul+reduce
                    # (tensor_tensor_reduce wedges the device)
                    nc.vector.tensor_mul(wsc[:], vr[:], q_bc[:])
                    nc.vector.reduce_sum(w_col[:, mc, :], wsc[:],
                                         axis=mybir.AxisListType.X)
            # vt_g[d2%128, j, d2c, m%128] <- [128, 4*512] (c = j*4+d2c)
            nc.sync.dma_start(vt_g[:], vr_g[:], transpose=True)
            vrg.append(vr_g)
            vtg.append(vt_g)
        return vrg, vtg, w_col

    def prep_audio(b, nt):
        n0 = b * SEQ + nt * NTW
        # audio slice -> fp16 (contiguous [128, 2048]) -> aT via ONE XBAR
        # transpose: aT[d1%128, rc, d1c, n%128] (c = rc*4+d1c)
        ab = acast.tile([P, NSC, D], F16, tag="acast")
        for rc in range(NSC):
            ar = araw.tile([P, D], F32, tag="araw")
            nc.scalar.dma_start(ar[:], audio[n0 + rc * P:n0 + (rc + 1) * P, :])
            nc.vector.tensor_copy(ab[:, rc, :], ar[:])
        aT = nt_pool.tile([P, NSC, DC, P], F16, tag="aT")
        nc.sync.dma_start(aT[:], ab[:], transpose=True)
        return aT

    # batch-0 input prefetch, issued before any setup compute can stall
    # the in-order ACT/DVE streams (has_ba: w_col needs q_bc from setup,
    # so the biased variant prefetches after setup instead)
    if not has_ba:
        vrg0, vtg0, w_col0 = prep_video(0)
        aT00 = prep_audio(0, 0)

    # ---- setup compute: M = Wa^T Wv (fp16), WoT, bias bcast ----
    wa_h = setup.tile([P, DC, D], F16, tag="wa_h")
    wv_h = setup.tile([P, DC, D], F16, tag="wv_h")
    for ec in range(DC):
        nc.scalar.copy(wa_h[:, ec, :], wa_sb[:, ec, :])
        nc.scalar.copy(wv_h[:, ec, :], wv_sb[:, ec, :])

    for d1c in range(DC):
        pm = ps_big.tile([P, NTW], F32, tag="big")
        for ec in range(DC):
            nc.tensor.matmul(pm[:], wa_h[:, ec, d1c * P:(d1c + 1) * P],
                             wv_h[:, ec, :], start=(ec == 0),
                             stop=(ec == DC - 1))
        nc.scalar.copy(m_sb[:, d1c, :], pm[:])

    wo_h = setup.tile([P, DC, D], F16, tag="wo_h")
    nc.vector.tensor_copy(wo_h[:], wo_sb[:])
    # woT[e%128, fc, ec, f%128] = Wo[fc*128+f', ec*128+e']: ONE XBAR
    # transpose of the [128, 2048] fp16 tile (c = fc*4+ec chunk-major)
    nc.sync.dma_start(woT[:], wo_h[:], transpose=True)

    bo_h = setup.tile([1, D], F16, tag="bo_h")
    nc.vector.tensor_copy(bo_h[:], bo_sb[:])
    pb = ps_big.tile([P, NTW], F32, tag="big")
    nc.tensor.matmul(pb[:], ones_row[:], bo_h[:], start=True, stop=True)
    nc.vector.tensor_copy(bo_bc[:], pb[:])

    if has_ba:
        ba_sb = setup.tile([P, DC], F32, tag="ba_sb")
        nc.sync.dma_start(ba_sb[:], nc.t_ba.ap().rearrange("(c p) -> p c", p=P))
        ba_r = setup.tile([P, DC], F16, tag="ba_r")
        nc.vector.tensor_copy(ba_r[:], ba_sb[:])
        # q = Wv^T b_a [1, d], pre-scaled by 1/sqrt(d); bcast to [128, d]
        pq = ps_big.tile([P, NTW], F32, tag="big")
        for ec in range(DC):
            nc.tensor.matmul(pq[0:1, :], ba_r[:, ec:ec + 1], wv_h[:, ec, :],
                             start=(ec == 0), stop=(ec == DC - 1))
        q_row = setup.tile([1, D], F16, tag="q_row")
        nc.scalar.mul(q_row[:], pq[0:1, :], SCALE)
        pqb = ps_big.tile([P, NTW], F32, tag="big")
        nc.tensor.matmul(pqb[:], ones_row[:], q_row[:], start=True, stop=True)
        nc.vector.tensor_copy(q_bc[:], pqb[:])

    if has_ba:
        vrg0, vtg0, w_col0 = prep_video(0)
        aT00 = prep_audio(0, 0)

    if KMODE < 4:
        # bisect modes: cover the output via DRAM->DRAM copy; stage tiles
        # below are additionally DMA'd over parts of it to defeat DCE
        nc.sync.dma_start(out[:, :], audio[:, :])
    if KMODE == 1:
        return

    for b in range(BL):
        b0 = b * SEQ
        if b == 0:
            vrg, vtg, w_col = vrg0, vtg0, w_col0
        else:
            vrg, vtg, w_col = prep_video(b)

        for nt in range(NT):
            n0 = b0 + nt * NTW
            aT = aT00 if (b == 0 and nt == 0) else prep_audio(b, nt)
            gT = nt_pool.tile([P, DC, NTW], F16, tag="gT")
            for d2c in range(DC):
                pg = ps_big.tile([P, NTW], F32, tag="big")
                for d1c in range(DC):
                    nc.tensor.matmul(pg[:], m_sb[:, d1c, d2c * P:(d2c + 1) * P],
                                     aT[:, :, d1c, :],
                                     start=(d1c == 0), stop=(d1c == DC - 1))
                nc.scalar.copy(gT[:, d2c, :], pg[:])

            if KMODE == 2:
                nc.sync.dma_start(out[n0:n0 + P, 0:NTW // 2],
                                  gT[:, 0, :].bitcast(F32))
                continue

            # scores -> exp (fp16) -> partial rowsums on DVE
            exp_t = exp_pool.tile([P, MC, NTW], F16, tag="exp_t")
            acc = accp.tile([P, NTW], F16, tag="acc")
            for mc in range(MC):
                psc = ps_big.tile([P, NTW], F32, tag="big")
                for d2c in range(DC):
                    nc.tensor.matmul(psc[:], vtg[mc // MG][:, mc % MG, d2c, :],
                                     gT[:, d2c, :],
                                     start=(d2c == 0), stop=(d2c == DC - 1))
                nc.scalar.activation(exp_t[:, mc, :], psc[:],
                                     mybir.ActivationFunctionType.Exp,
                                     bias=(w_col[:, mc, :] if has_ba else 0.0),
                                     scale=SCALE)
                if mc == 0:
                    nc.vector.tensor_copy(acc[:], exp_t[:, mc, :])
                else:
                    nc.vector.tensor_add(acc[:], acc[:], exp_t[:, mc, :])

            # rowsum columns: N=1 matmul per n-subchunk, then reciprocal
            recip_col = small.tile([P, NSC, 1], F32, tag="recip_col")
            for ns in range(NSC):
                prc = ps_rs.tile([P, 1], F32, tag="rs")
                nc.tensor.matmul(prc[:], acc[:, ns * P:(ns + 1) * P],
                                 ones_col[:], start=True, stop=True)
                nc.vector.reciprocal(recip_col[:, ns, :], prc[:])

            if KMODE == 3:
                nc.sync.dma_start(out[n0:n0 + P, 0:NTW // 2],
                                  exp_t[:, 0, :].bitcast(F32))
                nc.sync.dma_start(out[n0 + P:n0 + P + 1, 0:NSC],
                                  recip_col[0:1, :, 0])
                continue

            # att_T
            att_sb = nt_pool.tile([P, DC, NTW], F16, tag="att_sb")
            for dc in range(DC):
                pa = ps_big.tile([P, NTW], F32, tag="big")
                for mc in range(MC):
                    nc.tensor.matmul(pa[:],
                                     vrg[mc // MG][:, mc % MG,
                                                   dc * P:(dc + 1) * P],
                                     exp_t[:, mc, :],
                                     start=(mc == 0), stop=(mc == MC - 1))
                nc.vector.tensor_copy(att_sb[:, dc, :], pa[:])

            # out projection, normalize on ACT evict, bias on DVE
            for ns in range(NSC):
                po = ps_big.tile([P, D], F32, tag="big")
                for ec in range(DC):
                    nc.tensor.matmul(po[:], att_sb[:, ec, ns * P:(ns + 1) * P],
                                     woT[:, :, ec, :], start=(ec == 0),
                                     stop=(ec == DC - 1))
                o_sb = outp.tile([P, D], F32, tag="o_sb")
                nc.scalar.activation(o_sb[:], po[:],
                                     mybir.ActivationFunctionType.Copy,
                                     scale=recip_col[:, ns, :])
                nc.vector.tensor_add(o_sb[:], o_sb[:], bo_bc[:])
                nc.sync.dma_start(out[n0 + ns * P:n0 + (ns + 1) * P, :], o_sb[:])


_NC_CACHE = {}


def _build(has_ba=False):
    if has_ba in _NC_CACHE:
        return _NC_CACHE[has_ba]
    nc = bacc.Bacc("TRN2", target_bir_lowering=False, debug=False,
                   num_devices=NCORES)
    nc.t_audio = nc.dram_tensor("audio", [BL * SEQ, D], F32, kind="ExternalInput")
    nc.t_video = nc.dram_tensor("video", [BL * SEQ, D], F32, kind="ExternalInput")
    nc.t_wa = nc.dram_tensor("w_a", [D, D], F32, kind="ExternalInput")
    nc.t_wv = nc.dram_tensor("w_v", [D, D], F32, kind="ExternalInput")
    nc.t_wo = nc.dram_tensor("w_o", [D, D], F32, kind="ExternalInput")
    nc.t_ba = nc.dram_tensor("b_a", [D], F32, kind="ExternalInput")
    nc.t_bo = nc.dram_tensor("b_o", [D], F32, kind="ExternalInput")
    nc.t_out = nc.dram_tensor("out", [BL * SEQ, D], F32, kind="ExternalOutput")
    with tile.TileContext(nc) as tc:
        with ExitStack() as ctx:
            _body(tc, ctx, has_ba=has_ba)
    nc.compile()
    _NC_CACHE[has_ba] = nc
    return nc


def kernel(audio, video, W_a, b_a, W_v, b_v, W_o, b_o, _trace=False):
    nc = _build(has_ba=bool(np.any(np.asarray(b_a))))
    audio = np.ascontiguousarray(audio, dtype=np.float32)
    video = np.ascontiguousarray(video, dtype=np.float32)
    shared = {
        "w_a": np.ascontiguousarray(W_a, dtype=np.float32),
        "w_v": np.ascontiguousarray(W_v, dtype=np.float32),
        "w_o": np.ascontiguousarray(W_o, dtype=np.float32),
        "b_a": np.ascontiguousarray(b_a, dtype=np.float32),
        "b_o": np.ascontiguousarray(b_o, dtype=np.float32),
    }
    in_maps = []
    for c in range(NCORES):
        sl = slice(c * BL, (c + 1) * BL)
        in_maps.append({
            "audio": audio[sl].reshape(BL * SEQ, D),
            "video": video[sl].reshape(BL * SEQ, D),
            **shared,
        })
    res = run_bass_kernel_spmd(nc, in_maps, core_ids=list(range(NCORES)),
                               trace=_trace)
    out = np.concatenate(
        [res.results[c]["out"].reshape(BL, SEQ, D) for c in range(NCORES)],
        axis=0)
    if _trace:
        kernel.last_exec_time_ns = res.exec_time_ns
        kernel.last_results = res
    return out


# revision 15
# speedup vs baseline: 1.0255x; 1.0255x over previous
"""CrossModalAttention kernel for 8 Trainium2 NeuronCores.

Data-parallel over batch: B=16 -> 2 batches per core.

Math (per batch, with A=audio [N,d], B=video [M,d]):
  scores*sqrt(d) = (A Wa^T + 1 b_a^T)(B Wv^T + 1 b_v^T)^T
                 = A M B^T + (row-constant terms) + 1_n w^T
  with M = Wa^T Wv, w = B (Wv^T b_a).  Row-constant terms drop inside
  softmax, and max-subtraction is skipped: scores are ~N(0,1), far from
  fp16/fp32 exp overflow.
  attn     = exp_s / rowsum, with exp_s kept transposed [m, n]
  att_T[d,n] = sum_m video[m,d] exp_s[m,n]
  out[n,f] = (att_T^T @ Wo^T) * (1/rowsum[n]) + b_o

All matmul operands are fp16 (1 cyc/row on PE like bf16, fp32 PSUM
accumulation, 8x less rounding than bf16); softmax internals stay fp32.
K is always on partitions.  All transposes run on the DMA XBAR (2-byte
dtype, ~14ns per 16x128 tile), so the PE does zero transpose work.
Video is prepped in 4 groups of 4 m-chunks, each group an independent
load->cast->XBAR chain (tile-granular WAR deps would serialize a
monolithic chain).  Batch-0 video / audio-nt0 loads issue BEFORE the
weight-setup compute: the ACT sequencer is in-order, so any setup wait
would stall the load issue behind it.

  aT[d1,n]  <- one XBAR transpose of the [128, 4*512] fp16 audio tile
  gT[d2,n]  =  M[d1,d2](st)       x aT(mv)
  sT[m,n]   =  videoT[d2,m](st)   x gT(mv);   exp on ACT (fp16 out)
  acc[p,n]  += exp[p + 128*mc, n]             (DVE partial rowsum)
  rs[n,1]   =  acc[p,nslice](st)  x ones[p,1](mv)   (N=1 matmul)
  attT[d,n] =  video_r[m,d](st)   x exp(mv)
  out[n,f]  =  attT[e,n](st)      x WoT[e,f](mv); *recip on ACT evict
"""

import os
from contextlib import ExitStack

import numpy as np

# Stage bisect: 1=setup+copyout, 2=+audio prep+gT, 3=+scores/exp/rs,
# 4=full (default)
KMODE = int(os.environ.get("KMODE", "4"))

import concourse.bass as bass
import concourse.mybir as mybir
import concourse.tile as tile
from concourse import bacc
from concourse.bass_utils import run_bass_kernel_spmd

B, SEQ, D = 16, 2048, 512
NCORES = 8
BL = B // NCORES          # batches per core
P = 128
DC = D // P               # 4 chunks of the model dim
MC = SEQ // P             # 16 m-chunks per batch
VG = 4                    # video prep groups
MG = MC // VG             # m-chunks per group
NTW = 512                 # n-tile width
NT = SEQ // NTW           # 4 n-tiles per batch
NSC = NTW // P            # 4 n-subchunks per n-tile
SCALE = 1.0 / float(np.sqrt(D))

F32 = mybir.dt.float32
F16 = mybir.dt.float16
FR = mybir.dt.float32r


def _body(tc, ctx, has_ba=False):
    nc = tc.nc
    audio = nc.t_audio.ap()
    video = nc.t_video.ap()
    out = nc.t_out.ap()

    const = ctx.enter_context(tc.tile_pool(name="const", bufs=1))
    ps_big = ctx.enter_context(tc.tile_pool(name="ps_big", bufs=6, space="PSUM"))
    ps_rs = ctx.enter_context(tc.tile_pool(name="ps_rs", bufs=2, space="PSUM"))
    setup = ctx.enter_context(tc.tile_pool(name="setup", bufs=1))
    vid = ctx.enter_context(tc.tile_pool(name="vid", bufs=2))
    vraw = ctx.enter_context(tc.tile_pool(name="vraw", bufs=4))
    araw = ctx.enter_context(tc.tile_pool(name="araw", bufs=4))
    acast = ctx.enter_context(tc.tile_pool(name="acast", bufs=2))
    nt_pool = ctx.enter_context(tc.tile_pool(name="nt", bufs=2))
    exp_pool = ctx.enter_context(tc.tile_pool(name="expp", bufs=2))
    accp = ctx.enter_context(tc.tile_pool(name="accp", bufs=2))
    outp = ctx.enter_context(tc.tile_pool(name="outp", bufs=4))
    small = ctx.enter_context(tc.tile_pool(name="small", bufs=2))

    # ---- constants (no input deps) ----
    ones_f32 = const.tile([P, P], F32, tag="ones_f32")
    nc.gpsimd.memset(ones_f32[:], 1.0)
    ones_col = const.tile([P, 1], F16, tag="ones_col")
    nc.vector.tensor_copy(ones_col[:], ones_f32[:, 0:1])
    ones_row = const.tile([1, P], F16, tag="ones_row")
    nc.vector.tensor_copy(ones_row[:], ones_f32[0:1, :])

    m_sb = const.tile([P, DC, D], F16, tag="m_sb")
    woT = const.tile([P, DC, DC, P], F16, tag="woT")
    bo_bc = const.tile([P, D], F32, tag="bo_bc")
    q_bc = const.tile([P, D], F32, tag="q_bc") if has_ba else None

    # ---- issue weight DMAs early (ACT ring: only 4 slots, so the setup
    # casts right behind them on the in-order ACT stream are not delayed
    # by bulk-load ring backpressure) ----
    wa_sb = setup.tile([P, DC, D], F32, tag="wa_sb")
    wv_sb = setup.tile([P, DC, D], F32, tag="wv_sb")
    wo_sb = setup.tile([P, DC, D], F32, tag="wo_sb")
    bo_sb = setup.tile([1, D], F32, tag="bo_sb")
    nc.scalar.dma_start(wa_sb[:], nc.t_wa.ap().rearrange("(c p) d -> p c d", p=P))
    nc.scalar.dma_start(wv_sb[:], nc.t_wv.ap().rearrange("(c p) d -> p c d", p=P))
    nc.scalar.dma_start(wo_sb[:], nc.t_wo.ap().rearrange("(c p) d -> p c d", p=P))
    nc.scalar.dma_start(bo_sb[:], nc.t_bo.ap().rearrange("(o d) -> o d", o=1))

    # ---- video prep: 4 groups, each an independent load/cast/XBAR chain.
    # eng picks the DMA issue ring: batch-0 prefetch goes on SP so the
    # bounded-depth ring backpressure of 20 bulk loads cannot stall the
    # ACT sequencer (weights + setup casts + exp live there); steady-state
    # batches load via ACT, keeping SP free for stores + transposes. ----
    def prep_video(b, eng):
        b0 = b * SEQ
        vrg, vtg = [], []
        if has_ba:
            w_col = vid.tile([P, MC, 1], F32, tag="w_col")
            wsc = vid.tile([P, D], F32, tag="wsc")
        else:
            w_col = None
        for g in range(VG):
            vr_g = vid.tile([P, MG, D], F16, tag=f"vr{g}")
            vt_g = vid.tile([P, MG, DC, P], F16, tag=f"vt{g}")
            for j in range(MG):
                mc = g * MG + j
                vr = vraw.tile([P, D], F32, tag="vraw")
                eng.dma_start(vr[:], video[b0 + mc * P:b0 + (mc + 1) * P, :])
                nc.vector.tensor_copy(vr_g[:, j, :], vr[:])
                if has_ba:
                    # w = (video @ q)*scale via mul+reduce
                    # (tensor_tensor_reduce wedges the device)
                    nc.vector.tensor_mul(wsc[:], vr[:], q_bc[:])
                    nc.vector.reduce_sum(w_col[:, mc, :], wsc[:],
                                         axis=mybir.AxisListType.X)
            # vt_g[d2%128, j, d2c, m%128] <- [128, 4*512] (c = j*4+d2c)
            nc.sync.dma_start(vt_g[:], vr_g[:], transpose=True)
            vrg.append(vr_g)
            vtg.append(vt_g)
        return vrg, vtg, w_col

    def prep_audio(b, nt, eng):
        n0 = b * SEQ + nt * NTW
        # audio slice -> fp16 (contiguous [128, 2048]) -> aT via ONE XBAR
        # transpose: aT[d1%128, rc, d1c, n%128] (c = rc*4+d1c)
        ab = acast.tile([P, NSC, D], F16, tag="acast")
        for rc in range(NSC):
            ar = araw.tile([P, D], F32, tag="araw")
            eng.dma_start(ar[:], audio[n0 + rc * P:n0 + (rc + 1) * P, :])
            nc.vector.tensor_copy(ab[:, rc, :], ar[:])
        aT = nt_pool.tile([P, NSC, DC, P], F16, tag="aT")
        nc.sync.dma_start(aT[:], ab[:], transpose=True)
        return aT

    # batch-0 input prefetch on the SP ring: audio nt0 first (gT is the
    # first dependent PE work after M), then the video groups (has_ba:
    # w_col needs q_bc from setup, so the biased variant prefetches after
    # setup instead)
    if not has_ba:
        aT00 = prep_audio(0, 0, nc.sync)
        vrg0, vtg0, w_col0 = prep_video(0, nc.sync)

    # ---- setup compute: M = Wa^T Wv (fp16), WoT, bias bcast ----
    wa_h = setup.tile([P, DC, D], F16, tag="wa_h")
    wv_h = setup.tile([P, DC, D], F16, tag="wv_h")
    for ec in range(DC):
        nc.scalar.copy(wa_h[:, ec, :], wa_sb[:, ec, :])
        nc.scalar.copy(wv_h[:, ec, :], wv_sb[:, ec, :])

    for d1c in range(DC):
        pm = ps_big.tile([P, NTW], F32, tag="big")
        for ec in range(DC):
            nc.tensor.matmul(pm[:], wa_h[:, ec, d1c * P:(d1c + 1) * P],
                             wv_h[:, ec, :], start=(ec == 0),
                             stop=(ec == DC - 1))
        nc.scalar.copy(m_sb[:, d1c, :], pm[:])

    wo_h = setup.tile([P, DC, D], F16, tag="wo_h")
    nc.vector.tensor_copy(wo_h[:], wo_sb[:])
    # woT[e%128, fc, ec, f%128] = Wo[fc*128+f', ec*128+e']: ONE XBAR
    # transpose of the [128, 2048] fp16 tile (c = fc*4+ec chunk-major)
    nc.sync.dma_start(woT[:], wo_h[:], transpose=True)

    bo_h = setup.tile([1, D], F16, tag="bo_h")
    nc.vector.tensor_copy(bo_h[:], bo_sb[:])
    pb = ps_big.tile([P, NTW], F32, tag="big")
    nc.tensor.matmul(pb[:], ones_row[:], bo_h[:], start=True, stop=True)
    nc.vector.tensor_copy(bo_bc[:], pb[:])

    if has_ba:
        ba_sb = setup.tile([P, DC], F32, tag="ba_sb")
        nc.sync.dma_start(ba_sb[:], nc.t_ba.ap().rearrange("(c p) -> p c", p=P))
        ba_r = setup.tile([P, DC], F16, tag="ba_r")
        nc.vector.tensor_copy(ba_r[:], ba_sb[:])
        # q = Wv^T b_a [1, d], pre-scaled by 1/sqrt(d); bcast to [128, d]
        pq = ps_big.tile([P, NTW], F32, tag="big")
        for ec in range(DC):
            nc.tensor.matmul(pq[0:1, :], ba_r[:, ec:ec + 1], wv_h[:, ec, :],
                             start=(ec == 0), stop=(ec == DC - 1))
        q_row = setup.tile([1, D], F16, tag="q_row")
        nc.scalar.mul(q_row[:], pq[0:1, :], SCALE)
        pqb = ps_big.tile([P, NTW], F32, tag="big")
        nc.tensor.matmul(pqb[:], ones_row[:], q_row[:], start=True, stop=True)
        nc.vector.tensor_copy(q_bc[:], pqb[:])

    if has_ba:
        aT00 = prep_audio(0, 0, nc.sync)
        vrg0, vtg0, w_col0 = prep_video(0, nc.sync)

    if KMODE < 4:
        # bisect modes: cover the output via DRAM->DRAM copy; stage tiles
        # below are additionally DMA'd over parts of it to defeat DCE
        nc.sync.dma_start(out[:, :], audio[:, :])
    if KMODE == 1:
        return

    for b in range(BL):
        b0 = b * SEQ
        if b == 0:
            vrg, vtg, w_col = vrg0, vtg0, w_col0
        else:
            vrg, vtg, w_col = prep_video(b, nc.scalar)

        for nt in range(NT):
            n0 = b0 + nt * NTW
            aT = aT00 if (b == 0 and nt == 0) else prep_audio(b, nt, nc.scalar)
            gT = nt_pool.tile([P, DC, NTW], F16, tag="gT")
            for d2c in range(DC):
                pg = ps_big.tile([P, NTW], F32, tag="big")
                for d1c in range(DC):
                    nc.tensor.matmul(pg[:], m_sb[:, d1c, d2c * P:(d2c + 1) * P],
                                     aT[:, :, d1c, :],
                                     start=(d1c == 0), stop=(d1c == DC - 1))
                nc.scalar.copy(gT[:, d2c, :], pg[:])

            if KMODE == 2:
                nc.sync.dma_start(out[n0:n0 + P, 0:NTW // 2],
                                  gT[:, 0, :].bitcast(F32))
                continue

            # scores -> exp (fp16) -> partial rowsums on DVE
            exp_t = exp_pool.tile([P, MC, NTW], F16, tag="exp_t")
            acc = accp.tile([P, NTW], F16, tag="acc")
            for mc in range(MC):
                psc = ps_big.tile([P, NTW], F32, tag="big")
                for d2c in range(DC):
                    nc.tensor.matmul(psc[:], vtg[mc // MG][:, mc % MG, d2c, :],
                                     gT[:, d2c, :],
                                     start=(d2c == 0), stop=(d2c == DC - 1))
                nc.scalar.activation(exp_t[:, mc, :], psc[:],
                                     mybir.ActivationFunctionType.Exp,
                                     bias=(w_col[:, mc, :] if has_ba else 0.0),
                                     scale=SCALE)
                if mc == 0:
                    nc.vector.tensor_copy(acc[:], exp_t[:, mc, :])
                else:
                    nc.vector.tensor_add(acc[:], acc[:], exp_t[:, mc, :])

            # rowsum columns: N=1 matmul per n-subchunk, then reciprocal
            recip_col = small.tile([P, NSC, 1], F32, tag="recip_col")
            for ns in range(NSC):
                prc = ps_rs.tile([P, 1], F32, tag="rs")
                nc.tensor.matmul(prc[:], acc[:, ns * P:(ns + 1) * P],
                                 ones_col[:], start=True, stop=True)
                nc.vector.reciprocal(recip_col[:, ns, :], prc[:])

            if KMODE == 3:
                nc.sync.dma_start(out[n0:n0 + P, 0:NTW // 2],
                                  exp_t[:, 0, :].bitcast(F32))
                nc.sync.dma_start(out[n0 + P:n0 + P + 1, 0:NSC],
                                  recip_col[0:1, :, 0])
                continue

            # att_T
            att_sb = nt_pool.tile([P, DC, NTW], F16, tag="att_sb")
            for dc in range(DC):
                pa = ps_big.tile([P, NTW], F32, tag="big")
                for mc in range(MC):
                    nc.tensor.matmul(pa[:],
                                     vrg[mc // MG][:, mc % MG,
                                                   dc * P:(dc + 1) * P],
                                     exp_t[:, mc, :],
                                     start=(mc == 0), stop=(mc == MC - 1))
                nc.vector.tensor_copy(att_sb[:, dc, :], pa[:])

            # out projection, normalize on ACT evict, bias on DVE
            for ns in range(NSC):
                po = ps_big.tile([P, D], F32, tag="big")
                for ec in range(DC):
                    nc.tensor.matmul(po[:], att_sb[:, ec, ns * P:(ns + 1) * P],
                                     woT[:, :, ec, :], start=(ec == 0),
                                     stop=(ec == DC - 1))
                o_sb = outp.tile([P, D], F32, tag="o_sb")
                nc.scalar.activation(o_sb[:], po[:],
                                     mybir.ActivationFunctionType.Copy,
                                     scale=recip_col[:, ns, :])
                nc.vector.tensor_add(o_sb[:], o_sb[:], bo_bc[:])
                nc.sync.dma_start(out[n0 + ns * P:n0 + (ns + 1) * P, :], o_sb[:])


_NC_CACHE = {}


def _build(has_ba=False):
    if has_ba in _NC_CACHE:
        return _NC_CACHE[has_ba]
    nc = bacc.Bacc("TRN2", target_bir_lowering=False, debug=False,
                   num_devices=NCORES)
    nc.t_audio = nc.dram_tensor("audio", [BL * SEQ, D], F32, kind="ExternalInput")
    nc.t_video = nc.dram_tensor("video", [BL * SEQ, D], F32, kind="ExternalInput")
    nc.t_wa = nc.dram_tensor("w_a", [D, D], F32, kind="ExternalInput")
    nc.t_wv = nc.dram_tensor("w_v", [D, D], F32, kind="ExternalInput")
    nc.t_wo = nc.dram_tensor("w_o", [D, D], F32, kind="ExternalInput")
    nc.t_ba = nc.dram_tensor("b_a", [D], F32, kind="ExternalInput")
    nc.t_bo = nc.dram_tensor("b_o", [D], F32, kind="ExternalInput")
    nc.t_out = nc.dram_tensor("out", [BL * SEQ, D], F32, kind="ExternalOutput")
    with tile.TileContext(nc) as tc:
        with ExitStack() as ctx:
            _body(tc, ctx, has_ba=has_ba)
    nc.compile()
    _NC_CACHE[has_ba] = nc
    return nc


def kernel(audio, video, W_a, b_a, W_v, b_v, W_o, b_o, _trace=False):
    nc = _build(has_ba=bool(np.any(np.asarray(b_a))))
    audio = np.ascontiguousarray(audio, dtype=np.float32)
    video = np.ascontiguousarray(video, dtype=np.float32)
    shared = {
        "w_a": np.ascontiguousarray(W_a, dtype=np.float32),
        "w_v": np.ascontiguousarray(W_v, dtype=np.float32),
        "w_o": np.ascontiguousarray(W_o, dtype=np.float32),
        "b_a": np.ascontiguousarray(b_a, dtype=np.float32),
        "b_o": np.ascontiguousarray(b_o, dtype=np.float32),
    }
    in_maps = []
    for c in range(NCORES):
        sl = slice(c * BL, (c + 1) * BL)
        in_maps.append({
            "audio": audio[sl].reshape(BL * SEQ, D),
            "video": video[sl].reshape(BL * SEQ, D),
            **shared,
        })
    res = run_bass_kernel_spmd(nc, in_maps, core_ids=list(range(NCORES)),
                               trace=_trace)
    out = np.concatenate(
        [res.results[c]["out"].reshape(BL, SEQ, D) for c in range(NCORES)],
        axis=0)
    if _trace:
        kernel.last_exec_time_ns = res.exec_time_ns
        kernel.last_results = res
    return out


# revision 23
# speedup vs baseline: 1.0560x; 1.0297x over previous
"""CrossModalAttention kernel for 8 Trainium2 NeuronCores.

Data-parallel over batch: B=16 -> 2 batches per core.

Math (per batch, with A=audio [N,d], B=video [M,d]):
  scores*sqrt(d) = (A Wa^T + 1 b_a^T)(B Wv^T + 1 b_v^T)^T
                 = A M B^T + (row-constant terms) + 1_n w^T
  with M = Wa^T Wv, w = B (Wv^T b_a).  Row-constant terms drop inside
  softmax, and max-subtraction is skipped: scores are ~N(0,1), far from
  fp16/fp32 exp overflow.
  attn     = exp_s / rowsum, with exp_s kept transposed [m, n]
  att_T[d,n] = sum_m video[m,d] exp_s[m,n]
  out[n,f] = (att_T^T @ Wo^T) * (1/rowsum[n]) + b_o

All matmul operands are fp16 (1 cyc/row on PE like bf16, fp32 PSUM
accumulation, 8x less rounding than bf16); softmax internals stay fp32.
K is always on partitions.  All transposes run on the DMA XBAR (2-byte
dtype, ~14ns per 16x128 tile), so the PE does zero transpose work.
Video is prepped in 4 groups of 4 m-chunks, each group an independent
load->cast->XBAR chain (tile-granular WAR deps would serialize a
monolithic chain).  Batch-0 video / audio-nt0 loads issue BEFORE the
weight-setup compute: the ACT sequencer is in-order, so any setup wait
would stall the load issue behind it.

  aT[d1,n]  <- one XBAR transpose of the [128, 4*512] fp16 audio tile
  gT[d2,n]  =  M[d1,d2](st)       x aT(mv)
  sT[m,n]   =  videoT[d2,m](st)   x gT(mv);   exp on ACT (fp16 out)
  acc[p,n]  += exp[p + 128*mc, n]             (DVE partial rowsum)
  rs[n,1]   =  acc[p,nslice](st)  x ones[p,1](mv)   (N=1 matmul)
  attT[d,n] =  video_r[m,d](st)   x exp(mv)
  out[n,f]  =  attT[e,n](st)      x WoT[e,f](mv); *recip on ACT evict
"""

import os
from contextlib import ExitStack

import numpy as np

# Stage bisect: 1=setup+copyout, 2=+audio prep+gT, 3=+scores/exp/rs,
# 4=full (default)
KMODE = int(os.environ.get("KMODE", "4"))

import concourse.bass as bass
import concourse.mybir as mybir
import concourse.tile as tile
from concourse import bacc
from concourse.bass_utils import run_bass_kernel_spmd
from concourse.masks import make_identity

B, SEQ, D = 16, 2048, 512
NCORES = 8
BL = B // NCORES          # batches per core
P = 128
DC = D // P               # 4 chunks of the model dim
MC = SEQ // P             # 16 m-chunks per batch
VG = 4                    # video prep groups
MG = MC // VG             # m-chunks per group
NTW = 512                 # n-tile width
NT = SEQ // NTW           # 4 n-tiles per batch
NSC = NTW // P            # 4 n-subchunks per n-tile
SCALE = 1.0 / float(np.sqrt(D))

F32 = mybir.dt.float32
F16 = mybir.dt.float16
FR = mybir.dt.float32r


def _body(tc, ctx, has_ba=False):
    nc = tc.nc
    audio = nc.t_audio.ap()
    video = nc.t_video.ap()
    out = nc.t_out.ap()

    const = ctx.enter_context(tc.tile_pool(name="const", bufs=1))
    ps_big = ctx.enter_context(tc.tile_pool(name="ps_big", bufs=5, space="PSUM"))
    ps_tp = ctx.enter_context(tc.tile_pool(name="ps_tp", bufs=2, space="PSUM"))
    ps_rs = ctx.enter_context(tc.tile_pool(name="ps_rs", bufs=1, space="PSUM"))
    setup = ctx.enter_context(tc.tile_pool(name="setup", bufs=1))
    vid = ctx.enter_context(tc.tile_pool(name="vid", bufs=2))
    vraw = ctx.enter_context(tc.tile_pool(name="vraw", bufs=4))
    araw = ctx.enter_context(tc.tile_pool(name="araw", bufs=4))
    acast = ctx.enter_context(tc.tile_pool(name="acast", bufs=2))
    nt_pool = ctx.enter_context(tc.tile_pool(name="nt", bufs=2))
    exp_pool = ctx.enter_context(tc.tile_pool(name="expp", bufs=2))
    accp = ctx.enter_context(tc.tile_pool(name="accp", bufs=2))
    outp = ctx.enter_context(tc.tile_pool(name="outp", bufs=4))
    small = ctx.enter_context(tc.tile_pool(name="small", bufs=2))

    # ---- constants (no input deps) ----
    ones_f32 = const.tile([P, P], F32, tag="ones_f32")
    nc.gpsimd.memset(ones_f32[:], 1.0)
    ones_col = const.tile([P, 1], F16, tag="ones_col")
    nc.vector.tensor_copy(ones_col[:], ones_f32[:, 0:1])
    ones_row = const.tile([1, P], F16, tag="ones_row")
    nc.vector.tensor_copy(ones_row[:], ones_f32[0:1, :])
    ident = const.tile([P, P], F16, tag="ident")
    make_identity(nc, ident[:])

    m_sb = const.tile([P, DC, D], F16, tag="m_sb")
    woT = const.tile([P, DC, DC, P], F16, tag="woT")
    bo_bc = const.tile([P, D], F32, tag="bo_bc")
    if has_ba:
        q_bc = const.tile([P, D], F32, tag="q_bc", name="q_bc")
    else:
        q_bc = None

    # ---- issue weight DMAs early (ACT ring: only 4 slots, so the setup
    # casts right behind them on the in-order ACT stream are not delayed
    # by bulk-load ring backpressure) ----
    wa_sb = setup.tile([P, DC, D], F32, tag="wa_sb")
    wv_sb = setup.tile([P, DC, D], F32, tag="wv_sb")
    wo_sb = setup.tile([P, DC, D], F32, tag="wo_sb")
    bo_sb = setup.tile([1, D], F32, tag="bo_sb")
    nc.scalar.dma_start(wa_sb[:], nc.t_wa.ap().rearrange("(c p) d -> p c d", p=P))
    nc.scalar.dma_start(wv_sb[:], nc.t_wv.ap().rearrange("(c p) d -> p c d", p=P))
    nc.scalar.dma_start(wo_sb[:], nc.t_wo.ap().rearrange("(c p) d -> p c d", p=P))
    nc.scalar.dma_start(bo_sb[:], nc.t_bo.ap().rearrange("(o d) -> o d", o=1))

    # ---- video prep: 4 groups, each an independent load/cast/XBAR chain.
    # eng picks the DMA issue ring: batch-0 prefetch goes on SP so the
    # bounded-depth ring backpressure of 20 bulk loads cannot stall the
    # ACT sequencer (weights + setup casts + exp live there); steady-state
    # batches load via ACT, keeping SP free for stores + transposes. ----
    def prep_video(b, eng):
        b0 = b * SEQ
        vrg, vtg = [], []
        if has_ba:
            w_col = vid.tile([P, MC, 1], F32, tag="w_col")
            wsc = vid.tile([P, D], F32, tag="wsc")
        else:
            w_col = None
        for g in range(VG):
            vr_g = vid.tile([P, MG, D], F16, tag=f"vr{g}")
            vt_g = vid.tile([P, MG, DC, P], F16, tag=f"vt{g}")
            for j in range(MG):
                mc = g * MG + j
                vr = vraw.tile([P, D], F32, tag="vraw")
                eng.dma_start(vr[:], video[b0 + mc * P:b0 + (mc + 1) * P, :])
                nc.vector.tensor_copy(vr_g[:, j, :], vr[:])
                if has_ba:
                    # w = (video @ q)*scale via mul+reduce
                    # (tensor_tensor_reduce wedges the device)
                    nc.vector.tensor_mul(wsc[:], vr[:], q_bc[:])
                    nc.vector.reduce_sum(w_col[:, mc, :], wsc[:],
                                         axis=mybir.AxisListType.X)
            # vt_g[d2%128, j, d2c, m%128] <- [128, 4*512] (c = j*4+d2c)
            nc.sync.dma_start(vt_g[:], vr_g[:], transpose=True)
            vrg.append(vr_g)
            vtg.append(vt_g)
        return vrg, vtg, w_col

    def prep_audio(b, nt, eng):
        n0 = b * SEQ + nt * NTW
        # audio slice -> fp16 (contiguous [128, 2048]) -> aT via ONE XBAR
        # transpose: aT[d1%128, rc, d1c, n%128] (c = rc*4+d1c)
        ab = acast.tile([P, NSC, D], F16, tag="acast")
        for rc in range(NSC):
            ar = araw.tile([P, D], F32, tag="araw")
            eng.dma_start(ar[:], audio[n0 + rc * P:n0 + (rc + 1) * P, :])
            nc.vector.tensor_copy(ab[:, rc, :], ar[:])
        aT = nt_pool.tile([P, NSC, DC, P], F16, tag="aT")
        nc.sync.dma_start(aT[:], ab[:], transpose=True)
        return aT

    # batch-0 input prefetch on the SP ring: loads + fp16 casts ONLY.  The
    # transposes for batch 0 run on the PE (idle at startup; fp16 transpose
    # is 1 cyc/row), interleaved into the nt0 instruction stream below —
    # XBAR-transpose descriptors would head-of-line block the load queues
    # and complete ~30us late.  (has_ba: w_col needs q_bc from setup, so
    # the biased variant prefetches after setup via the XBAR path instead.)
    if not has_ba:
        ab0 = acast.tile([P, NSC, D], F16, tag="acast")
        for rc in range(NSC):
            ar = araw.tile([P, D], F32, tag="araw")
            nc.sync.dma_start(ar[:], audio[rc * P:(rc + 1) * P, :])
            nc.vector.tensor_copy(ab0[:, rc, :], ar[:])
        vrg0, vtg0 = [], []
        for g in range(VG):
            vr_g = vid.tile([P, MG, D], F16, tag=f"vr{g}")
            vt_g = vid.tile([P, MG, DC, P], F16, tag=f"vt{g}")
            for j in range(MG):
                mc = g * MG + j
                vr = vraw.tile([P, D], F32, tag="vraw")
                nc.sync.dma_start(vr[:], video[mc * P:(mc + 1) * P, :])
                nc.vector.tensor_copy(vr_g[:, j, :], vr[:])
            vrg0.append(vr_g)
            vtg0.append(vt_g)
        w_col0 = None

    # ---- setup compute: M = Wa^T Wv (fp16), WoT, bias bcast ----
    wa_h = setup.tile([P, DC, D], F16, tag="wa_h")
    wv_h = setup.tile([P, DC, D], F16, tag="wv_h")
    for ec in range(DC):
        nc.scalar.copy(wa_h[:, ec, :], wa_sb[:, ec, :])
        nc.scalar.copy(wv_h[:, ec, :], wv_sb[:, ec, :])

    for d1c in range(DC):
        pm = ps_big.tile([P, NTW], F32, tag="big")
        for ec in range(DC):
            nc.tensor.matmul(pm[:], wa_h[:, ec, d1c * P:(d1c + 1) * P],
                             wv_h[:, ec, :], start=(ec == 0),
                             stop=(ec == DC - 1))
        nc.scalar.copy(m_sb[:, d1c, :], pm[:])

    wo_h = setup.tile([P, DC, D], F16, tag="wo_h")
    nc.vector.tensor_copy(wo_h[:], wo_sb[:])
    # woT[e%128, fc, ec, f%128] = Wo[fc*128+f', ec*128+e']: ONE XBAR
    # transpose of the [128, 2048] fp16 tile (c = fc*4+ec chunk-major)
    nc.sync.dma_start(woT[:], wo_h[:], transpose=True)

    bo_h = setup.tile([1, D], F16, tag="bo_h")
    nc.vector.tensor_copy(bo_h[:], bo_sb[:])
    pb = ps_big.tile([P, NTW], F32, tag="big")
    nc.tensor.matmul(pb[:], ones_row[:], bo_h[:], start=True, stop=True)
    nc.vector.tensor_copy(bo_bc[:], pb[:])

    if has_ba:
        ba_sb = setup.tile([P, DC], F32, tag="ba_sb")
        nc.sync.dma_start(ba_sb[:], nc.t_ba.ap().rearrange("(c p) -> p c", p=P))
        ba_r = setup.tile([P, DC], F16, tag="ba_r")
        nc.vector.tensor_copy(ba_r[:], ba_sb[:])
        # q = Wv^T b_a [1, d], pre-scaled by 1/sqrt(d); bcast to [128, d]
        pq = ps_big.tile([P, NTW], F32, tag="big")
        for ec in range(DC):
            nc.tensor.matmul(pq[0:1, :], ba_r[:, ec:ec + 1], wv_h[:, ec, :],
                             start=(ec == 0), stop=(ec == DC - 1))
        q_row = setup.tile([1, D], F16, tag="q_row")
        nc.scalar.mul(q_row[:], pq[0:1, :], SCALE)
        pqb = ps_big.tile([P, NTW], F32, tag="big")
        nc.tensor.matmul(pqb[:], ones_row[:], q_row[:], start=True, stop=True)
        nc.vector.tensor_copy(q_bc[:], pqb[:])

    if has_ba:
        aT00 = prep_audio(0, 0, nc.sync)
        vrg0, vtg0, w_col0 = prep_video(0, nc.sync)
        pe_prep0 = False
    else:
        pe_prep0 = True

    if KMODE < 4:
        # bisect modes: cover the output via DRAM->DRAM copy; stage tiles
        # below are additionally DMA'd over parts of it to defeat DCE
        nc.sync.dma_start(out[:, :], audio[:, :])
    if KMODE == 1:
        return

    def pe_transpose(dst, src):
        # [128,128] fp16 transpose through the PE (dst/src SBUF slices)
        pt = ps_tp.tile([P, P], F16, tag="tp")
        nc.tensor.transpose(pt[:], src, ident[:])
        nc.vector.tensor_copy(dst, pt[:])

    for b in range(BL):
        b0 = b * SEQ
        pe_prep = pe_prep0 and b == 0
        if b == 0:
            vrg, vtg, w_col = vrg0, vtg0, w_col0
        else:
            vrg, vtg, w_col = prep_video(b, nc.scalar)

        for nt in range(NT):
            n0 = b0 + nt * NTW
            if pe_prep and nt == 0:
                aT = nt_pool.tile([P, NSC, DC, P], F16, tag="aT")
                for rc in range(NSC):
                    for dc in range(DC):
                        pe_transpose(aT[:, rc, dc, :],
                                     ab0[:, rc, dc * P:(dc + 1) * P])
            elif b == 0 and nt == 0:
                aT = aT00
            else:
                aT = prep_audio(b, nt, nc.scalar)
            gT = nt_pool.tile([P, DC, NTW], F16, tag="gT")
            for d2c in range(DC):
                pg = ps_big.tile([P, NTW], F32, tag="big")
                for d1c in range(DC):
                    nc.tensor.matmul(pg[:], m_sb[:, d1c, d2c * P:(d2c + 1) * P],
                                     aT[:, :, d1c, :],
                                     start=(d1c == 0), stop=(d1c == DC - 1))
                nc.scalar.copy(gT[:, d2c, :], pg[:])

            if KMODE == 2:
                nc.sync.dma_start(out[n0:n0 + P, 0:NTW // 2],
                                  gT[:, 0, :].bitcast(F32))
                continue

            # scores -> exp (fp16) -> partial rowsums on DVE
            exp_t = exp_pool.tile([P, MC, NTW], F16, tag="exp_t")
            acc = accp.tile([P, NTW], F16, tag="acc")
            for mc in range(MC):
                if pe_prep and nt == 0 and mc % MG == 0:
                    # transpose video group g on the PE right before the
                    # first scores chunk that consumes it
                    g = mc // MG
                    for j in range(MG):
                        for dc in range(DC):
                            pe_transpose(vtg[g][:, j, dc, :],
                                         vrg[g][:, j, dc * P:(dc + 1) * P])
                psc = ps_big.tile([P, NTW], F32, tag="big")
                for d2c in range(DC):
                    nc.tensor.matmul(psc[:], vtg[mc // MG][:, mc % MG, d2c, :],
                                     gT[:, d2c, :],
                                     start=(d2c == 0), stop=(d2c == DC - 1))
                nc.scalar.activation(exp_t[:, mc, :], psc[:],
                                     mybir.ActivationFunctionType.Exp,
                                     bias=(w_col[:, mc, :] if has_ba else 0.0),
                                     scale=SCALE)
                if mc == 0:
                    nc.vector.tensor_copy(acc[:], exp_t[:, mc, :])
                else:
                    nc.vector.tensor_add(acc[:], acc[:], exp_t[:, mc, :])

            # rowsum columns: N=1 matmul per n-subchunk, then reciprocal
            recip_col = small.tile([P, NSC, 1], F32, tag="recip_col")
            for ns in range(NSC):
                prc = ps_rs.tile([P, 1], F32, tag="rs")
                nc.tensor.matmul(prc[:], acc[:, ns * P:(ns + 1) * P],
                                 ones_col[:], start=True, stop=True)
                nc.vector.reciprocal(recip_col[:, ns, :], prc[:])

            if KMODE == 3:
                nc.sync.dma_start(out[n0:n0 + P, 0:NTW // 2],
                                  exp_t[:, 0, :].bitcast(F32))
                nc.sync.dma_start(out[n0 + P:n0 + P + 1, 0:NSC],
                                  recip_col[0:1, :, 0])
                continue

            # att_T
            att_sb = nt_pool.tile([P, DC, NTW], F16, tag="att_sb")
            for dc in range(DC):
                pa = ps_big.tile([P, NTW], F32, tag="big")
                for mc in range(MC):
                    nc.tensor.matmul(pa[:],
                                     vrg[mc // MG][:, mc % MG,
                                                   dc * P:(dc + 1) * P],
                                     exp_t[:, mc, :],
                                     start=(mc == 0), stop=(mc == MC - 1))
                nc.vector.tensor_copy(att_sb[:, dc, :], pa[:])

            # out projection, normalize on ACT evict, bias on DVE
            for ns in range(NSC):
                po = ps_big.tile([P, D], F32, tag="big")
                for ec in range(DC):
                    nc.tensor.matmul(po[:], att_sb[:, ec, ns * P:(ns + 1) * P],
                                     woT[:, :, ec, :], start=(ec == 0),
                                     stop=(ec == DC - 1))
                o_sb = outp.tile([P, D], F32, tag="o_sb")
                nc.scalar.activation(o_sb[:], po[:],
                                     mybir.ActivationFunctionType.Copy,
                                     scale=recip_col[:, ns, :])
                nc.vector.tensor_add(o_sb[:], o_sb[:], bo_bc[:])
                nc.sync.dma_start(out[n0 + ns * P:n0 + (ns + 1) * P, :], o_sb[:])


_NC_CACHE = {}


def _build(has_ba=False):
    if has_ba in _NC_CACHE:
        return _NC_CACHE[has_ba]
    nc = bacc.Bacc("TRN2", target_bir_lowering=False, debug=False,
                   num_devices=NCORES)
    nc.t_audio = nc.dram_tensor("audio", [BL * SEQ, D], F32, kind="ExternalInput")
    nc.t_video = nc.dram_tensor("video", [BL * SEQ, D], F32, kind="ExternalInput")
    nc.t_wa = nc.dram_tensor("w_a", [D, D], F32, kind="ExternalInput")
    nc.t_wv = nc.dram_tensor("w_v", [D, D], F32, kind="ExternalInput")
    nc.t_wo = nc.dram_tensor("w_o", [D, D], F32, kind="ExternalInput")
    nc.t_ba = nc.dram_tensor("b_a", [D], F32, kind="ExternalInput")
    nc.t_bo = nc.dram_tensor("b_o", [D], F32, kind="ExternalInput")
    nc.t_out = nc.dram_tensor("out", [BL * SEQ, D], F32, kind="ExternalOutput")
    with tile.TileContext(nc) as tc:
        with ExitStack() as ctx:
            _body(tc, ctx, has_ba=has_ba)
    nc.compile()
    _NC_CACHE[has_ba] = nc
    return nc


def kernel(audio, video, W_a, b_a, W_v, b_v, W_o, b_o, _trace=False):
    nc = _build(has_ba=bool(np.any(np.asarray(b_a))))
    audio = np.ascontiguousarray(audio, dtype=np.float32)
    video = np.ascontiguousarray(video, dtype=np.float32)
    shared = {
        "w_a": np.ascontiguousarray(W_a, dtype=np.float32),
        "w_v": np.ascontiguousarray(W_v, dtype=np.float32),
        "w_o": np.ascontiguousarray(W_o, dtype=np.float32),
        "b_a": np.ascontiguousarray(b_a, dtype=np.float32),
        "b_o": np.ascontiguousarray(b_o, dtype=np.float32),
    }
    in_maps = []
    for c in range(NCORES):
        sl = slice(c * BL, (c + 1) * BL)
        in_maps.append({
            "audio": audio[sl].reshape(BL * SEQ, D),
            "video": video[sl].reshape(BL * SEQ, D),
            **shared,
        })
    res = run_bass_kernel_spmd(nc, in_maps, core_ids=list(range(NCORES)),
                               trace=_trace)
    out = np.concatenate(
        [res.results[c]["out"].reshape(BL, SEQ, D) for c in range(NCORES)],
        axis=0)
    if _trace:
        kernel.last_exec_time_ns = res.exec_time_ns
        kernel.last_results = res
    return out


# revision 25
# speedup vs baseline: 1.0695x; 1.0128x over previous
"""CrossModalAttention kernel for 8 Trainium2 NeuronCores.

Data-parallel over batch: B=16 -> 2 batches per core.

Math (per batch, with A=audio [N,d], B=video [M,d]):
  scores*sqrt(d) = (A Wa^T + 1 b_a^T)(B Wv^T + 1 b_v^T)^T
                 = A M B^T + (row-constant terms) + 1_n w^T
  with M = Wa^T Wv, w = B (Wv^T b_a).  Row-constant terms drop inside
  softmax, and max-subtraction is skipped: scores are ~N(0,1), far from
  fp16/fp32 exp overflow.
  attn     = exp_s / rowsum, with exp_s kept transposed [m, n]
  att_T[d,n] = sum_m video[m,d] exp_s[m,n]
  out[n,f] = (att_T^T @ Wo^T) * (1/rowsum[n]) + b_o

All matmul operands are fp16 (1 cyc/row on PE like bf16, fp32 PSUM
accumulation, 8x less rounding than bf16); softmax internals stay fp32.
K is always on partitions.  All transposes run on the DMA XBAR (2-byte
dtype, ~14ns per 16x128 tile), so the PE does zero transpose work.
Video is prepped in 4 groups of 4 m-chunks, each group an independent
load->cast->XBAR chain (tile-granular WAR deps would serialize a
monolithic chain).  Batch-0 video / audio-nt0 loads issue BEFORE the
weight-setup compute: the ACT sequencer is in-order, so any setup wait
would stall the load issue behind it.

  aT[d1,n]  <- one XBAR transpose of the [128, 4*512] fp16 audio tile
  gT[d2,n]  =  M[d1,d2](st)       x aT(mv)
  sT[m,n]   =  videoT[d2,m](st)   x gT(mv);   exp on ACT (fp16 out)
  acc[p,n]  += exp[p + 128*mc, n]             (DVE partial rowsum)
  rs[n,1]   =  acc[p,nslice](st)  x ones[p,1](mv)   (N=1 matmul)
  attT[d,n] =  video_r[m,d](st)   x exp(mv)
  out[n,f]  =  attT[e,n](st)      x WoT[e,f](mv); *recip on ACT evict
"""

import os
from contextlib import ExitStack

import numpy as np

# Stage bisect: 1=setup+copyout, 2=+audio prep+gT, 3=+scores/exp/rs,
# 4=full (default)
KMODE = int(os.environ.get("KMODE", "4"))

import concourse.bass as bass
import concourse.mybir as mybir
import concourse.tile as tile
from concourse import bacc
from concourse.bass_utils import run_bass_kernel_spmd
from concourse.masks import make_identity

B, SEQ, D = 16, 2048, 512
NCORES = 8
BL = B // NCORES          # batches per core
P = 128
DC = D // P               # 4 chunks of the model dim
MC = SEQ // P             # 16 m-chunks per batch
VG = 4                    # video prep groups
MG = MC // VG             # m-chunks per group
NTW = 512                 # n-tile width
NT = SEQ // NTW           # 4 n-tiles per batch
NSC = NTW // P            # 4 n-subchunks per n-tile
SCALE = 1.0 / float(np.sqrt(D))

F32 = mybir.dt.float32
F16 = mybir.dt.float16
FR = mybir.dt.float32r


def _body(tc, ctx, has_ba=False):
    nc = tc.nc
    audio = nc.t_audio.ap()
    video = nc.t_video.ap()
    out = nc.t_out.ap()

    const = ctx.enter_context(tc.tile_pool(name="const", bufs=1))
    ps_big = ctx.enter_context(tc.tile_pool(name="ps_big", bufs=5, space="PSUM"))
    ps_tp = ctx.enter_context(tc.tile_pool(name="ps_tp", bufs=2, space="PSUM"))
    ps_rs = ctx.enter_context(tc.tile_pool(name="ps_rs", bufs=1, space="PSUM"))
    setup = ctx.enter_context(tc.tile_pool(name="setup", bufs=1))
    vid = ctx.enter_context(tc.tile_pool(name="vid", bufs=2))
    # has_ba adds q_bc/wsc/w_col tiles; shrink rotating pools to fit SBUF
    rb = 3 if has_ba else 4
    vraw = ctx.enter_context(tc.tile_pool(name="vraw", bufs=rb))
    araw = ctx.enter_context(tc.tile_pool(name="araw", bufs=rb))
    acast = ctx.enter_context(tc.tile_pool(name="acast", bufs=2))
    nt_pool = ctx.enter_context(tc.tile_pool(name="nt", bufs=2))
    exp_pool = ctx.enter_context(tc.tile_pool(name="expp", bufs=2))
    accp = ctx.enter_context(tc.tile_pool(name="accp", bufs=2))
    outp = ctx.enter_context(tc.tile_pool(name="outp", bufs=rb))
    small = ctx.enter_context(tc.tile_pool(name="small", bufs=2))

    # ---- constants (no input deps) ----
    ones_f32 = const.tile([P, P], F32, tag="ones_f32")
    nc.gpsimd.memset(ones_f32[:], 1.0)
    ones_col = const.tile([P, 1], F16, tag="ones_col")
    nc.vector.tensor_copy(ones_col[:], ones_f32[:, 0:1])
    ones_row = const.tile([1, P], F16, tag="ones_row")
    nc.vector.tensor_copy(ones_row[:], ones_f32[0:1, :])
    ident = const.tile([P, P], F16, tag="ident")
    make_identity(nc, ident[:])

    m_sb = const.tile([P, DC, D], F16, tag="m_sb")
    woT = const.tile([P, DC, DC, P], F16, tag="woT")
    bo_bc = const.tile([P, D], F32, tag="bo_bc")
    if has_ba:
        q_bc = const.tile([P, D], F32, tag="q_bc", name="q_bc")
    else:
        q_bc = None

    # ---- issue weight DMAs early (ACT ring: only 4 slots, so the setup
    # casts right behind them on the in-order ACT stream are not delayed
    # by bulk-load ring backpressure) ----
    wa_sb = setup.tile([P, DC, D], F32, tag="wa_sb")
    wv_sb = setup.tile([P, DC, D], F32, tag="wv_sb")
    wo_sb = setup.tile([P, DC, D], F32, tag="wo_sb")
    bo_sb = setup.tile([1, D], F32, tag="bo_sb")
    nc.scalar.dma_start(wa_sb[:], nc.t_wa.ap().rearrange("(c p) d -> p c d", p=P))
    nc.scalar.dma_start(wv_sb[:], nc.t_wv.ap().rearrange("(c p) d -> p c d", p=P))
    nc.scalar.dma_start(wo_sb[:], nc.t_wo.ap().rearrange("(c p) d -> p c d", p=P))
    nc.scalar.dma_start(bo_sb[:], nc.t_bo.ap().rearrange("(o d) -> o d", o=1))

    # ---- video prep: 4 groups, each an independent load/cast/XBAR chain.
    # eng picks the DMA issue ring: batch-0 prefetch goes on SP so the
    # bounded-depth ring backpressure of 20 bulk loads cannot stall the
    # ACT sequencer (weights + setup casts + exp live there); steady-state
    # batches load via ACT, keeping SP free for stores + transposes. ----
    def prep_video(b, eng):
        b0 = b * SEQ
        vrg, vtg = [], []
        if has_ba:
            w_col = vid.tile([P, MC, 1], F32, tag="w_col")
            wsc = vid.tile([P, D], F32, tag="wsc")
        else:
            w_col = None
        for g in range(VG):
            vr_g = vid.tile([P, MG, D], F16, tag=f"vr{g}")
            vt_g = vid.tile([P, MG, DC, P], F16, tag=f"vt{g}")
            for j in range(MG):
                mc = g * MG + j
                vr = vraw.tile([P, D], F32, tag="vraw")
                eng.dma_start(vr[:], video[b0 + mc * P:b0 + (mc + 1) * P, :])
                nc.vector.tensor_copy(vr_g[:, j, :], vr[:])
                if has_ba:
                    # w = (video @ q)*scale via mul+reduce
                    # (tensor_tensor_reduce wedges the device)
                    nc.vector.tensor_mul(wsc[:], vr[:], q_bc[:])
                    nc.vector.reduce_sum(w_col[:, mc, :], wsc[:],
                                         axis=mybir.AxisListType.X)
            # vt_g[d2%128, j, d2c, m%128] <- [128, 4*512] (c = j*4+d2c)
            nc.sync.dma_start(vt_g[:], vr_g[:], transpose=True)
            vrg.append(vr_g)
            vtg.append(vt_g)
        return vrg, vtg, w_col

    def prep_audio(b, nt, eng):
        n0 = b * SEQ + nt * NTW
        # audio slice -> fp16 (contiguous [128, 2048]) -> aT via ONE XBAR
        # transpose: aT[d1%128, rc, d1c, n%128] (c = rc*4+d1c)
        ab = acast.tile([P, NSC, D], F16, tag="acast")
        for rc in range(NSC):
            ar = araw.tile([P, D], F32, tag="araw")
            eng.dma_start(ar[:], audio[n0 + rc * P:n0 + (rc + 1) * P, :])
            nc.vector.tensor_copy(ab[:, rc, :], ar[:])
        aT = nt_pool.tile([P, NSC, DC, P], F16, tag="aT")
        nc.sync.dma_start(aT[:], ab[:], transpose=True)
        return aT

    # batch-0 input prefetch on the SP ring: loads + fp16 casts ONLY.  The
    # transposes for batch 0 run on the PE (idle at startup; fp16 transpose
    # is 1 cyc/row), interleaved into the nt0 instruction stream below —
    # XBAR-transpose descriptors would head-of-line block the load queues
    # and complete ~30us late.  (has_ba: w_col needs q_bc from setup, so
    # the biased variant prefetches after setup via the XBAR path instead.)
    if not has_ba:
        ab0 = acast.tile([P, NSC, D], F16, tag="acast")
        for rc in range(NSC):
            ar = araw.tile([P, D], F32, tag="araw")
            nc.sync.dma_start(ar[:], audio[rc * P:(rc + 1) * P, :])
            nc.vector.tensor_copy(ab0[:, rc, :], ar[:])
        vrg0, vtg0 = [], []
        for g in range(VG):
            vr_g = vid.tile([P, MG, D], F16, tag=f"vr{g}")
            vt_g = vid.tile([P, MG, DC, P], F16, tag=f"vt{g}")
            for j in range(MG):
                mc = g * MG + j
                vr = vraw.tile([P, D], F32, tag="vraw")
                nc.sync.dma_start(vr[:], video[mc * P:(mc + 1) * P, :])
                nc.vector.tensor_copy(vr_g[:, j, :], vr[:])
            vrg0.append(vr_g)
            vtg0.append(vt_g)
        w_col0 = None

    # ---- setup compute: M = Wa^T Wv (fp16), WoT, bias bcast ----
    wa_h = setup.tile([P, DC, D], F16, tag="wa_h")
    wv_h = setup.tile([P, DC, D], F16, tag="wv_h")
    for ec in range(DC):
        nc.scalar.copy(wa_h[:, ec, :], wa_sb[:, ec, :])
        nc.scalar.copy(wv_h[:, ec, :], wv_sb[:, ec, :])

    for d1c in range(DC):
        pm = ps_big.tile([P, NTW], F32, tag="big")
        for ec in range(DC):
            nc.tensor.matmul(pm[:], wa_h[:, ec, d1c * P:(d1c + 1) * P],
                             wv_h[:, ec, :], start=(ec == 0),
                             stop=(ec == DC - 1))
        nc.scalar.copy(m_sb[:, d1c, :], pm[:])

    wo_h = setup.tile([P, DC, D], F16, tag="wo_h")
    nc.vector.tensor_copy(wo_h[:], wo_sb[:])
    # woT[e%128, fc, ec, f%128] = Wo[fc*128+f', ec*128+e']: ONE XBAR
    # transpose of the [128, 2048] fp16 tile (c = fc*4+ec chunk-major)
    nc.sync.dma_start(woT[:], wo_h[:], transpose=True)

    bo_h = setup.tile([1, D], F16, tag="bo_h")
    nc.vector.tensor_copy(bo_h[:], bo_sb[:])
    pb = ps_big.tile([P, NTW], F32, tag="big")
    nc.tensor.matmul(pb[:], ones_row[:], bo_h[:], start=True, stop=True)
    nc.vector.tensor_copy(bo_bc[:], pb[:])

    if has_ba:
        ba_sb = setup.tile([P, DC], F32, tag="ba_sb")
        nc.sync.dma_start(ba_sb[:], nc.t_ba.ap().rearrange("(c p) -> p c", p=P))
        ba_r = setup.tile([P, DC], F16, tag="ba_r")
        nc.vector.tensor_copy(ba_r[:], ba_sb[:])
        # q = Wv^T b_a [1, d], pre-scaled by 1/sqrt(d); bcast to [128, d]
        pq = ps_big.tile([P, NTW], F32, tag="big")
        for ec in range(DC):
            nc.tensor.matmul(pq[0:1, :], ba_r[:, ec:ec + 1], wv_h[:, ec, :],
                             start=(ec == 0), stop=(ec == DC - 1))
        q_row = setup.tile([1, D], F16, tag="q_row")
        nc.scalar.mul(q_row[:], pq[0:1, :], SCALE)
        pqb = ps_big.tile([P, NTW], F32, tag="big")
        nc.tensor.matmul(pqb[:], ones_row[:], q_row[:], start=True, stop=True)
        nc.vector.tensor_copy(q_bc[:], pqb[:])

    if has_ba:
        aT00 = prep_audio(0, 0, nc.sync)
        vrg0, vtg0, w_col0 = prep_video(0, nc.sync)
        pe_prep0 = False
    else:
        pe_prep0 = True

    if KMODE < 4:
        # bisect modes: cover the output via DRAM->DRAM copy; stage tiles
        # below are additionally DMA'd over parts of it to defeat DCE
        nc.sync.dma_start(out[:, :], audio[:, :])
    if KMODE == 1:
        return

    def pe_transpose(dst, src):
        # [128,128] fp16 transpose through the PE (dst/src SBUF slices)
        pt = ps_tp.tile([P, P], F16, tag="tp")
        nc.tensor.transpose(pt[:], src, ident[:])
        nc.vector.tensor_copy(dst, pt[:])

    aT_next = None
    for b in range(BL):
        b0 = b * SEQ
        pe_prep = pe_prep0 and b == 0
        if b == 0:
            vrg, vtg, w_col = vrg0, vtg0, w_col0
        else:
            vrg, vtg, w_col = prep_video(b, nc.scalar)

        for nt in range(NT):
            n0 = b0 + nt * NTW
            if pe_prep and nt == 0:
                aT = nt_pool.tile([P, NSC, DC, P], F16, tag="aT")
                for rc in range(NSC):
                    for dc in range(DC):
                        pe_transpose(aT[:, rc, dc, :],
                                     ab0[:, rc, dc * P:(dc + 1) * P])
            elif b == 0 and nt == 0:
                aT = aT00
            else:
                aT = aT_next
            # prefetch the NEXT n-tile's audio one iteration ahead: its
            # load->cast->XBAR chain (~10us) then hides under this tile's
            # scores/attT instead of stalling the next gT
            nxt = b * NT + nt + 1
            if nxt < BL * NT:
                aT_next = prep_audio(nxt // NT, nxt % NT, nc.scalar)
            gT = nt_pool.tile([P, DC, NTW], F16, tag="gT")
            for d2c in range(DC):
                pg = ps_big.tile([P, NTW], F32, tag="big")
                for d1c in range(DC):
                    nc.tensor.matmul(pg[:], m_sb[:, d1c, d2c * P:(d2c + 1) * P],
                                     aT[:, :, d1c, :],
                                     start=(d1c == 0), stop=(d1c == DC - 1))
                nc.scalar.copy(gT[:, d2c, :], pg[:])

            if KMODE == 2:
                nc.sync.dma_start(out[n0:n0 + P, 0:NTW // 2],
                                  gT[:, 0, :].bitcast(F32))
                continue

            # scores -> exp (fp16) -> partial rowsums on DVE
            exp_t = exp_pool.tile([P, MC, NTW], F16, tag="exp_t")
            acc = accp.tile([P, NTW], F16, tag="acc")
            for mc in range(MC):
                if pe_prep and nt == 0 and mc % MG == 0:
                    # transpose video group g on the PE right before the
                    # first scores chunk that consumes it
                    g = mc // MG
                    for j in range(MG):
                        for dc in range(DC):
                            pe_transpose(vtg[g][:, j, dc, :],
                                         vrg[g][:, j, dc * P:(dc + 1) * P])
                psc = ps_big.tile([P, NTW], F32, tag="big")
                for d2c in range(DC):
                    nc.tensor.matmul(psc[:], vtg[mc // MG][:, mc % MG, d2c, :],
                                     gT[:, d2c, :],
                                     start=(d2c == 0), stop=(d2c == DC - 1))
                nc.scalar.activation(exp_t[:, mc, :], psc[:],
                                     mybir.ActivationFunctionType.Exp,
                                     bias=(w_col[:, mc, :] if has_ba else 0.0),
                                     scale=SCALE)
                if mc == 0:
                    nc.vector.tensor_copy(acc[:], exp_t[:, mc, :])
                else:
                    nc.vector.tensor_add(acc[:], acc[:], exp_t[:, mc, :])

            # rowsum columns: N=1 matmul per n-subchunk, then reciprocal
            recip_col = small.tile([P, NSC, 1], F32, tag="recip_col")
            for ns in range(NSC):
                prc = ps_rs.tile([P, 1], F32, tag="rs")
                nc.tensor.matmul(prc[:], acc[:, ns * P:(ns + 1) * P],
                                 ones_col[:], start=True, stop=True)
                nc.vector.reciprocal(recip_col[:, ns, :], prc[:])

            if KMODE == 3:
                nc.sync.dma_start(out[n0:n0 + P, 0:NTW // 2],
                                  exp_t[:, 0, :].bitcast(F32))
                nc.sync.dma_start(out[n0 + P:n0 + P + 1, 0:NSC],
                                  recip_col[0:1, :, 0])
                continue

            # att_T
            att_sb = nt_pool.tile([P, DC, NTW], F16, tag="att_sb")
            for dc in range(DC):
                pa = ps_big.tile([P, NTW], F32, tag="big")
                for mc in range(MC):
                    nc.tensor.matmul(pa[:],
                                     vrg[mc // MG][:, mc % MG,
                                                   dc * P:(dc + 1) * P],
                                     exp_t[:, mc, :],
                                     start=(mc == 0), stop=(mc == MC - 1))
                nc.vector.tensor_copy(att_sb[:, dc, :], pa[:])

            # out projection, normalize on ACT evict, bias on DVE
            for ns in range(NSC):
                po = ps_big.tile([P, D], F32, tag="big")
                for ec in range(DC):
                    nc.tensor.matmul(po[:], att_sb[:, ec, ns * P:(ns + 1) * P],
                                     woT[:, :, ec, :], start=(ec == 0),
                                     stop=(ec == DC - 1))
                o_sb = outp.tile([P, D], F32, tag="o_sb")
                nc.scalar.activation(o_sb[:], po[:],
                                     mybir.ActivationFunctionType.Copy,
                                     scale=recip_col[:, ns, :])
                nc.vector.tensor_add(o_sb[:], o_sb[:], bo_bc[:])
                nc.sync.dma_start(out[n0 + ns * P:n0 + (ns + 1) * P, :], o_sb[:])


_NC_CACHE = {}


def _build(has_ba=False):
    if has_ba in _NC_CACHE:
        return _NC_CACHE[has_ba]
    nc = bacc.Bacc("TRN2", target_bir_lowering=False, debug=False,
                   num_devices=NCORES)
    nc.t_audio = nc.dram_tensor("audio", [BL * SEQ, D], F32, kind="ExternalInput")
    nc.t_video = nc.dram_tensor("video", [BL * SEQ, D], F32, kind="ExternalInput")
    nc.t_wa = nc.dram_tensor("w_a", [D, D], F32, kind="ExternalInput")
    nc.t_wv = nc.dram_tensor("w_v", [D, D], F32, kind="ExternalInput")
    nc.t_wo = nc.dram_tensor("w_o", [D, D], F32, kind="ExternalInput")
    nc.t_ba = nc.dram_tensor("b_a", [D], F32, kind="ExternalInput")
    nc.t_bo = nc.dram_tensor("b_o", [D], F32, kind="ExternalInput")
    nc.t_out = nc.dram_tensor("out", [BL * SEQ, D], F32, kind="ExternalOutput")
    with tile.TileContext(nc) as tc:
        with ExitStack() as ctx:
            _body(tc, ctx, has_ba=has_ba)
    nc.compile()
    _NC_CACHE[has_ba] = nc
    return nc


def kernel(audio, video, W_a, b_a, W_v, b_v, W_o, b_o, _trace=False):
    nc = _build(has_ba=bool(np.any(np.asarray(b_a))))
    audio = np.ascontiguousarray(audio, dtype=np.float32)
    video = np.ascontiguousarray(video, dtype=np.float32)
    shared = {
        "w_a": np.ascontiguousarray(W_a, dtype=np.float32),
        "w_v": np.ascontiguousarray(W_v, dtype=np.float32),
        "w_o": np.ascontiguousarray(W_o, dtype=np.float32),
        "b_a": np.ascontiguousarray(b_a, dtype=np.float32),
        "b_o": np.ascontiguousarray(b_o, dtype=np.float32),
    }
    in_maps = []
    for c in range(NCORES):
        sl = slice(c * BL, (c + 1) * BL)
        in_maps.append({
            "audio": audio[sl].reshape(BL * SEQ, D),
            "video": video[sl].reshape(BL * SEQ, D),
            **shared,
        })
    res = run_bass_kernel_spmd(nc, in_maps, core_ids=list(range(NCORES)),
                               trace=_trace)
    out = np.concatenate(
        [res.results[c]["out"].reshape(BL, SEQ, D) for c in range(NCORES)],
        axis=0)
    if _trace:
        kernel.last_exec_time_ns = res.exec_time_ns
        kernel.last_results = res
    return out
